# revision 7
# baseline (speedup 1.0000x reference)
"""Transposed-orientation rewrite of the ON-LSTM kernel.

Differences vs kernel.py (baseline):
  * The recurrence runs "transposed": gate preactivations are computed as
    xoT[gate_col, batch] via weight-stationary matmuls (lhsT = 128x128
    weight block, rhs = hT/xT [128, 64]), all in bf16 (1 cycle/row on PE).
    h state is kept as hT [h_dim, batch] so the per-step PE transposes of
    h for the next matmul disappear.
  * The constant bias row cb is injected into each PSUM accumulation with a
    rank-1 broadcast matmul (cb_slice [1,128] x ones [1,64]); accumulation
    groups are kept contiguous per step (cross-step PSUM preload corrupts
    results on HW when two groups are open in one bank).
  * Per-(batch,l) cell coefficients (s1h, s2', s3') are broadcast across the
    128 partitions with rank-1 matmuls after a small transpose.
  * The sigmoid affine (0.5x+0.5) of the f/i gates is folded into the
    coefficients: F = s1*sig(f) + s2 = (0.5 s1) tanh + (s2 + 0.5 s1).
  * All inputs packed into 4 buffers (x, wt bf16, wf f32, gidx) to cut
    per-call dispatch marshalling.

Self-contained: hardcodes all shapes; no file reads.
"""

import numpy as np
import ml_dtypes

import concourse.bass as bass
import concourse.tile as tile
from concourse import mybir
from concourse.bass_utils import run_bass_kernel_spmd
from concourse.masks import make_identity

F32 = mybir.dt.float32
F32R = mybir.dt.float32r
BF16 = mybir.dt.bfloat16
I32 = mybir.dt.int32
AF = mybir.ActivationFunctionType
OP = mybir.AluOpType
AX = mybir.AxisListType

B, T, F, H, L, K, LAB = 512, 128, 256, 384, 3, 10, 25
CH = H // L            # 128
GATES = 4 * H + 2 * L  # 1542
NG = 1536
NCORES = 8
BL = B // NCORES       # 64
PAD = K - 1

# wt (bf16) column layout
WT_WK = 0              # 2 x [128,1536]
WT_WR = 3072           # 3 x [128,1536]
WT_WKS = 7680          # 2 x [128,6]
WT_WRS = 7692          # 3 x [128,6]
WT_CONV = 7710         # [128, 11520]
WT_CB = 19230          # row 0: [1,1536] cb (t>0)
WT_CB0 = 20766         # row 0: [1,1536] cb0 (t=0)
WT_CBS = 22302         # row 0: [1,6] slot cb, [1,6] slot cb0
WT_ONES = 22314        # row 0: [1,64] ones
WT_COLS = 22378

# wf (f32) column layout
WF_SCALEW = 0          # [128,192]
WF_RESCALEW = 192      # rows 0:64 [64,384]
WF_OUTW = 576          # [128,75]
WF_RESCB = 651         # [128,3]
WF_CONVB = 654         # [128,3]
WF_SCALEB = 657        # rows 0:64 [64,1]
WF_OUTB = 658          # row 0 [1,25]
WF_SEL = 684           # rows 0:16 [16, 9*128] row-selector matrices
WF_ONES = 1836         # row 0: [1,128] ones
WF_COLS = 1964


def _gate_perm_scale():
    # gate-major col-groups: [f0 f1 f2 | i0 i1 i2 | o0 o1 o2 | ci0 ci1 ci2]
    perm = np.zeros(GATES, np.int64)
    scale = np.ones(GATES, np.float32)
    for gi in range(4):   # f, i, o, ci
        for l in range(L):
            base = (gi * 3 + l) * CH
            perm[base: base + CH] = np.arange(
                2 * L + gi * H + l * CH, 2 * L + gi * H + (l + 1) * CH)
    scale[0: 3 * 3 * CH] = 0.5   # f, i, o via sigmoid-as-tanh
    perm[NG:] = np.arange(2 * L)
    return perm, scale


def _prep_shared(kernel_w, kernel_b, rec_w, rec_b, scale_w, scale_b,
                 rescale_w, rescale_b, conv_w, conv_b, out_w, out_b):
    perm, colscale = _gate_perm_scale()

    def reorder(v):
        return (v[..., perm] * colscale).astype(np.float32)

    wpre = reorder(kernel_w[:F])                     # [256, 1542]
    wrec = reorder(rec_w[:H])                        # [384, 1542]
    cb = reorder(kernel_b + rec_b + kernel_w[F] + rec_w[H])   # [1542]
    cb0 = reorder(kernel_b + rec_b)                  # [1542] (t=0: Tint=0)

    wt = np.zeros((128, WT_COLS), np.float32)
    for fc in range(2):
        wt[:, WT_WK + fc * NG: WT_WK + (fc + 1) * NG] = wpre[fc * 128:(fc + 1) * 128, :NG]
        wt[:, WT_WKS + fc * 6: WT_WKS + (fc + 1) * 6] = wpre[fc * 128:(fc + 1) * 128, NG:]
    for hc in range(3):
        wt[:, WT_WR + hc * NG: WT_WR + (hc + 1) * NG] = wrec[hc * 128:(hc + 1) * 128, :NG]
        wt[:, WT_WRS + hc * 6: WT_WRS + (hc + 1) * 6] = wrec[hc * 128:(hc + 1) * 128, NG:]
    # conv_w [O,Hin,K] -> [128(h'), (k,hc,oc,o)]
    for k in range(K):
        for hc in range(3):
            for oc in range(3):
                blk = conv_w[oc * CH:(oc + 1) * CH, hc * CH:(hc + 1) * CH, k].T
                wt[:, WT_CONV + (((k * 3 + hc) * 3 + oc) * CH):
                   WT_CONV + (((k * 3 + hc) * 3 + oc) * CH) + CH] = blk
    wt[0, WT_CB:WT_CB + NG] = cb[:NG]
    wt[0, WT_CB0:WT_CB0 + NG] = cb0[:NG]
    wt[0, WT_CBS:WT_CBS + 6] = cb[NG:]
    wt[0, WT_CBS + 6:WT_CBS + 12] = cb0[NG:]

    wf = np.zeros((128, WF_COLS), np.float32)
    for hcc in range(3):
        wf[:, WF_SCALEW + hcc * 64: WF_SCALEW + (hcc + 1) * 64] = \
            scale_w[hcc * CH:(hcc + 1) * CH, :] / 10.0
    wf[0:64, WF_RESCALEW:WF_RESCALEW + H] = rescale_w
    for oc in range(3):
        wf[:, WF_OUTW + oc * LAB: WF_OUTW + (oc + 1) * LAB] = out_w[oc * CH:(oc + 1) * CH, :]
        wf[:, WF_RESCB + oc] = 0.5 * rescale_b[oc * CH:(oc + 1) * CH]
        wf[:, WF_CONVB + oc] = conv_b[oc * CH:(oc + 1) * CH]
    wf[0:64, WF_SCALEB] = scale_b
    wf[0, WF_OUTB:WF_OUTB + LAB] = out_b
    for j in range(9):
        wf[j, WF_SEL + j * 128:WF_SEL + (j + 1) * 128] = 1.0
    wt[0, WT_ONES:WT_ONES + BL] = 1.0
    wf[0, WF_ONES:WF_ONES + 128] = 1.0

    return dict(wt=wt.astype(ml_dtypes.bfloat16), wf=wf)


def build_nc(t_steps=T, debug=False):
    nc = bass.Bass()
    ROWS = BL * t_steps
    HS_ROWS = (t_steps + PAD) * BL

    d_x = nc.dram_tensor("x", [BL, t_steps, F], F32, kind="ExternalInput")
    d_wt = nc.dram_tensor("wt", [128, WT_COLS], BF16, kind="ExternalInput")
    d_wf = nc.dram_tensor("wf", [128, WF_COLS], F32, kind="ExternalInput")
    d_gidx = nc.dram_tensor("gidx", [128, 5], I32, kind="ExternalInput")

    d_hseq = nc.dram_tensor("hseq", [HS_ROWS, H + 1], F32)
    d_out = nc.dram_tensor("cur_out", [BL, LAB], F32, kind="ExternalOutput")
    d_dscr = nc.dram_tensor("dscr", [1, K * BL], F32)
    d_dbg_ht = d_dbg_gath = d_dbg_dwin = None
    if debug:
        d_dbg_ht = nc.dram_tensor("dbg_ht", [t_steps * BL, H + 1], F32,
                                  kind="ExternalOutput")
        d_dbg_gath = nc.dram_tensor("dbg_gath", [128, H + 1], F32,
                                    kind="ExternalOutput")
        d_dbg_dwin = nc.dram_tensor("dbg_dwin", [BL, 16], F32,
                                    kind="ExternalOutput")

    with tile.TileContext(nc) as tc:
        with (
            tc.tile_pool(name="singles", bufs=1) as singles,
            tc.tile_pool(name="post", bufs=1) as post_p,
        ):
            ident = singles.tile([128, 128], F32)
            make_identity(nc, ident[:])
            id64 = ident[0:64, 0:64]
            wt_sb = singles.tile([128, WT_COLS], BF16)
            nc.sync.dma_start(wt_sb[:], d_wt[:])
            wf_sb = singles.tile([128, WF_COLS], F32)
            nc.sync.dma_start(wf_sb[:], d_wf[:])
            gidx_sb = singles.tile([128, 5], I32)
            nc.sync.dma_start(gidx_sb[:], d_gidx[:])
            ones_bf_t = wt_sb[0:1, WT_ONES:WT_ONES + BL]
            ones_fr = singles.tile([1, 128], F32R)
            nc.sync.dma_start(ones_fr[:],
                              d_wf[0:1, WF_ONES:WF_ONES + 128].bitcast(F32R))

            WK = [wt_sb[:, WT_WK + fc * NG: WT_WK + (fc + 1) * NG] for fc in range(2)]
            WR = [wt_sb[:, WT_WR + hc * NG: WT_WR + (hc + 1) * NG] for hc in range(3)]
            WKS = [wt_sb[:, WT_WKS + fc * 6: WT_WKS + (fc + 1) * 6] for fc in range(2)]
            WRS = [wt_sb[:, WT_WRS + hc * 6: WT_WRS + (hc + 1) * 6] for hc in range(3)]
            cb_row = [wt_sb[0:1, WT_CB:WT_CB + NG],
                      wt_sb[0:1, WT_CB0:WT_CB0 + NG]]
            cbs_row = [wt_sb[0:1, WT_CBS:WT_CBS + 6],
                       wt_sb[0:1, WT_CBS + 6:WT_CBS + 12]]
            sel_sb = singles.tile([16, 9 * 128], F32R)
            nc.sync.dma_start(sel_sb[:],
                              d_wf[0:16, WF_SEL:WF_SEL + 9 * 128].bitcast(F32R))
            sel = sel_sb[:]

            # zero hseq prefix rows
            zrow = singles.tile([128, H + 1], F32)
            nc.vector.memset(zrow[:], 0.0)
            for r0 in range(0, PAD * BL, 128):
                n = min(128, PAD * BL - r0)
                nc.sync.dma_start(d_hseq[r0:r0 + n, :], zrow[:n, :])

            # ---- phase 1: build XT (bf16, f-major) ----
            xt_sb = [singles.tile([128, ROWS], BF16, tag=f"xt{i}", name=f"xt{i}")
                     for i in range(2)]
            x_tmaj = d_x[:].rearrange("b t f -> t b f")
            with (
                tc.tile_pool(name="xrow", bufs=4) as xrow_p,
                tc.tile_pool(name="trps", bufs=4, space="PSUM") as trps_p,
            ):
                for rt in range(ROWS // 128):
                    xr = xrow_p.tile([128, F], F32, tag="xrow", name="xr")
                    t0 = rt * 2
                    nc.sync.dma_start(xr[0:64, :], x_tmaj[t0, :, :])
                    nc.sync.dma_start(xr[64:128, :], x_tmaj[t0 + 1, :, :])
                    for fc in range(2):
                        pt = trps_p.tile([128, 128], F32, tag="xtp", name="pt")
                        nc.tensor.transpose(pt[:], xr[:, fc * 128:(fc + 1) * 128],
                                            ident[:])
                        if fc == 0:
                            nc.scalar.copy(xt_sb[fc][:, rt * 128:(rt + 1) * 128], pt[:])
                        else:
                            nc.vector.tensor_copy(xt_sb[fc][:, rt * 128:(rt + 1) * 128],
                                                  pt[:])

            # ---- phase 2: recurrence ----
            with (
                tc.tile_pool(name="xo", bufs=2, space="PSUM") as xo_p,
                tc.tile_pool(name="smallps", bufs=1, space="PSUM") as smallps_p,
                tc.tile_pool(name="gates", bufs=2) as gates_p,
                tc.tile_pool(name="state", bufs=2) as state_p,
                tc.tile_pool(name="sm", bufs=2) as sm_p,
                tc.tile_pool(name="tmp", bufs=2) as tmp_p,
            ):
                slotP = smallps_p.tile([BL, 512], F32, tag="slotP", name="slotP")
                coefP = smallps_p.tile([128, 640], F32, tag="coefP", name="coefP")
                htP = smallps_p.tile([BL, 512], F32, tag="htP", name="htP")

                hT_prev = None
                cT_prev = None

                for t in range(t_steps):
                    brow = 1 if t == 0 else 0
                    ts = slice(t * BL, (t + 1) * BL)
                    xo_cur = xo_p.tile([128, 768], F32, tag="xoT", name=f"xo{t}")
                    sslot = slotP[:, (t % 64) * 8:(t % 64) * 8 + 6]

                    # -- PE: slot then gate groups, contiguous per group --
                    nc.tensor.matmul(sslot, ones_bf_t, cbs_row[brow],
                                     start=True, stop=False)
                    for fc in range(2):
                        nc.tensor.matmul(sslot, xt_sb[fc][:, ts], WKS[fc],
                                         start=False, stop=(t == 0 and fc == 1))
                    if t > 0:
                        for hc in range(3):
                            nc.tensor.matmul(sslot,
                                             hT_prev[:, hc * BL:(hc + 1) * BL],
                                             WRS[hc], start=False, stop=(hc == 2))
                    for g in range(12):
                        dst = xo_cur[:, g * 64:(g + 1) * 64]
                        nc.tensor.matmul(dst, cb_row[brow][:, g * 128:(g + 1) * 128],
                                         ones_bf_t, start=True, stop=False)
                        for fc in range(2):
                            nc.tensor.matmul(dst, WK[fc][:, g * 128:(g + 1) * 128],
                                             xt_sb[fc][:, ts],
                                             start=False, stop=(t == 0 and fc == 1))
                        if t > 0:
                            for hc in range(3):
                                nc.tensor.matmul(dst,
                                                 WR[hc][:, g * 128:(g + 1) * 128],
                                                 hT_prev[:, hc * BL:(hc + 1) * BL],
                                                 start=False, stop=(hc == 2))

                    # -- ACT/DVE: fm/im softmax chain -> s coefficients --
                    # fm = cum-l2r softmax(fm preact), im = cum-r2l.
                    # F = s1*sig(f)+s2, I = s1*sig(i)+s3 with s1 = fm*im,
                    # s2 = fm-s1, s3 = im-s1, sigmoid folded: s1h = 0.5*s1,
                    # s2' = s2+s1h = fm-s1h, s3' = im-s1h.
                    sm = sm_p.tile([BL, 32], F32, tag="sm", name="sm")
                    coefs = sm_p.tile([BL, 16], F32, tag="coefs", name="coefs")
                    nc.scalar.activation(sm[:, 0:6], sslot, AF.Exp)
                    nc.vector.tensor_reduce(
                        sm[:, 8:10], sm[:, 0:6].rearrange("p (a b) -> p a b", b=3),
                        axis=AX.X, op=OP.add)
                    nc.vector.tensor_tensor(sm[:, 1:2], sm[:, 0:1], sm[:, 1:2],
                                            op=OP.add)   # e0+e1
                    nc.vector.tensor_tensor(sm[:, 4:5], sm[:, 5:6], sm[:, 4:5],
                                            op=OP.add)   # e4+e5
                    nc.vector.reciprocal(sm[:, 10:12], sm[:, 8:10])
                    nc.vector.memset(sm[:, 14:16], 1.0)   # fm2 = 1, im0 = 1
                    nc.vector.memset(sm[:, 28:29], 0.5)   # imh0 = 0.5
                    nc.vector.tensor_scalar(sm[:, 12:14], sm[:, 0:2],
                                            scalar1=sm[:, 10:11], scalar2=None,
                                            op0=OP.mult)          # fm0, fm1
                    nc.vector.tensor_scalar(sm[:, 16:18], sm[:, 4:6],
                                            scalar1=sm[:, 11:12], scalar2=None,
                                            op0=OP.mult)          # im1, im2
                    nc.vector.tensor_scalar(sm[:, 29:31], sm[:, 4:6],
                                            scalar1=sm[:, 11:12], scalar2=0.5,
                                            op0=OP.mult, op1=OP.mult)  # imh1, imh2
                    nc.vector.tensor_tensor(coefs[:, 0:3], sm[:, 12:15],
                                            sm[:, 28:31], op=OP.mult)  # s1h
                    nc.vector.tensor_tensor(coefs[:, 3:6], sm[:, 12:15], coefs[:, 0:3],
                                            op=OP.subtract)  # s2' = fm - s1h
                    nc.vector.tensor_tensor(coefs[:, 6:9], sm[:, 15:18], coefs[:, 0:3],
                                            op=OP.subtract)  # s3' = im - s1h

                    # -- PE: transpose coefs, broadcast rows across partitions --
                    nc.tensor.transpose(coefP[0:9, 576:640], coefs[:, 0:9], id64)
                    coefT = sm_p.tile([9, 64], F32R, tag="coefT", name="coefT")
                    nc.vector.tensor_copy(coefT[:], coefP[0:9, 576:640])
                    for j in (0, 1, 2, 6, 7, 8, 3, 4, 5):
                        nc.tensor.matmul(coefP[:, j * 64:(j + 1) * 64],
                                         sel[0:9, j * 128:(j + 1) * 128],
                                         coefT[:], start=True, stop=True)

                    # -- ACT: gate tanh, one full-width instruction --
                    gatesT = gates_p.tile([128, 768], F32, tag="gatesT", name="gatesT")
                    nc.scalar.activation(gatesT[:], xo_cur[:], AF.Tanh)
                    thf = gatesT[:, 0:192]
                    thi = gatesT[:, 192:384]
                    tho = gatesT[:, 384:576]
                    ci = gatesT[:, 576:768]
                    s1h = coefP[:, 0:192]
                    s2p = coefP[:, 192:384]
                    s3p = coefP[:, 384:576]

                    # -- cell update, full-width [128,192] ops --
                    # c = s1h.(thi.ci + thf.c_prev) + (s3'.ci + s2'.c_prev)
                    # spine ops on DVE (258ns); Pool feeds the side products.
                    W3 = 3 * BL
                    a = tmp_p.tile([128, W3], F32, tag="a", name="a")
                    nc.vector.tensor_tensor(a[:], thi, ci, op=OP.mult)
                    if t > 0:
                        b_ = tmp_p.tile([128, W3], F32, tag="b", name="b_")
                        nc.gpsimd.tensor_tensor(b_[:], thf, cT_prev[:], op=OP.mult)
                        nc.vector.tensor_tensor(a[:], a[:], b_[:], op=OP.add)
                    m = tmp_p.tile([128, W3], F32, tag="m", name="m")
                    nc.vector.tensor_tensor(m[:], s1h, a[:], op=OP.mult)
                    e = tmp_p.tile([128, W3], F32, tag="e", name="e")
                    nc.vector.tensor_tensor(e[:], s3p, ci, op=OP.mult)
                    if t > 0:
                        f2 = tmp_p.tile([128, W3], F32, tag="f", name="f2")
                        nc.vector.tensor_tensor(f2[:], s2p, cT_prev[:], op=OP.mult)
                        nc.gpsimd.tensor_tensor(e[:], e[:], f2[:], op=OP.add)
                    c_new = state_p.tile([128, W3], F32, tag="c", name="c_new")
                    nc.vector.tensor_tensor(c_new[:], m[:], e[:], op=OP.add)
                    tc_ = tmp_p.tile([128, W3], F32, tag="tc", name="tc_")
                    nc.scalar.activation(tc_[:], c_new[:], AF.Tanh)
                    oaf = tmp_p.tile([128, W3], F32, tag="oaf", name="oaf")
                    nc.gpsimd.tensor_scalar(oaf[:], tho, scalar1=0.5, scalar2=0.5,
                                            op0=OP.mult, op1=OP.add)
                    hT_new = state_p.tile([128, W3], BF16, tag="hTn", name="hT_new")
                    nc.vector.tensor_tensor(hT_new[:], tc_[:], oaf[:], op=OP.mult)
                    hTf = state_p.tile([128, W3], F32, tag="hTf", name="hTf")
                    nc.gpsimd.tensor_tensor(hTf[:], tc_[:], oaf[:], op=OP.mult)

                    # -- dist for the window head --
                    ht_sb = sm_p.tile([BL, H + 1], F32, tag="ht_sb", name="ht_sb")
                    nc.vector.tensor_tensor(sm[:, 27:28], sm[:, 12:13], sm[:, 13:14],
                                            op=OP.add)
                    nc.vector.tensor_scalar(
                        ht_sb[:, H:H + 1], sm[:, 27:28],
                        scalar1=-1.0 / 3.0, scalar2=2.0 / 3.0, op0=OP.mult, op1=OP.add)

                    # -- PE: h transposes for the hseq store --
                    for l in range(3):
                        nc.tensor.transpose(htP[:, l * 128:(l + 1) * 128],
                                            hTf[:, l * BL:(l + 1) * BL], ident[:])
                    nc.vector.tensor_copy(ht_sb[:, 0:H], htP[:, 0:H])
                    nc.sync.dma_start(d_hseq[(t + PAD) * BL:(t + PAD + 1) * BL, :],
                                      ht_sb[:])
                    if debug:
                        nc.sync.dma_start(d_dbg_ht[t * BL:(t + 1) * BL, :], ht_sb[:])

                    hT_prev = hT_new[:]
                    cT_prev = c_new[:]

            # ---- phase 3: windowed head at t_b only (baseline logic) ----
            with (
                tc.tile_pool(name="postps", bufs=1, space="PSUM") as postps_p,
                tc.tile_pool(name="postps2", bufs=2, space="PSUM") as postps2_p,
            ):
                gath = [post_p.tile([128, H + 1], F32, tag=f"gath{j}",
                                    name=f"gath{j}") for j in range(5)]
                for j in range(5):
                    nc.gpsimd.indirect_dma_start(
                        out=gath[j][:], out_offset=None, in_=d_hseq[:],
                        in_offset=bass.IndirectOffsetOnAxis(ap=gidx_sb[:, j:j + 1],
                                                            axis=0))

                ww = post_p.tile([BL, 16], F32, name="ww")
                cum = post_p.tile([BL, 64], F32, name="cum")
                for k in range(K):
                    j, par = k // 2, k % 2
                    nc.vector.tensor_copy(ww[:, k:k + 1],
                                          gath[j][par * 64:par * 64 + 64, H:H + 1])
                nc.vector.tensor_copy(cum[:, 0:1], ww[:, 0:1])
                nc.vector.tensor_tensor(cum[:, 1:10], ww[:, 1:10], ww[:, 0:9],
                                        op=OP.add)
                nc.vector.tensor_copy(cum[:, 16:18], cum[:, 0:2])
                nc.vector.tensor_tensor(cum[:, 18:26], cum[:, 2:10], cum[:, 0:8],
                                        op=OP.add)
                nc.vector.tensor_copy(cum[:, 32:36], cum[:, 16:20])
                nc.vector.tensor_tensor(cum[:, 36:42], cum[:, 20:26], cum[:, 16:22],
                                        op=OP.add)
                nc.vector.tensor_copy(cum[:, 48:56], cum[:, 32:40])
                nc.vector.tensor_tensor(cum[:, 56:58], cum[:, 40:42], cum[:, 32:34],
                                        op=OP.add)
                nmx = post_p.tile([BL, 4], F32, name="nmx")
                nc.vector.tensor_reduce(nmx[:, 0:1], cum[:, 48:58], axis=AX.X,
                                        op=OP.max, negate=True)
                dwin = post_p.tile([BL, 16], F32, name="dwin")
                nc.scalar.activation(dwin[:, 0:10], cum[:, 48:58], AF.Exp,
                                     bias=nmx[:, 0:1], scale=1.0)
                nc.vector.tensor_reduce(nmx[:, 1:2], dwin[:, 0:10], axis=AX.X,
                                        op=OP.add)
                nc.vector.reciprocal(nmx[:, 2:3], nmx[:, 1:2])
                nc.vector.tensor_scalar(dwin[:, 0:10], dwin[:, 0:10],
                                        scalar1=nmx[:, 2:3], scalar2=None,
                                        op0=OP.mult)
                if debug:
                    nc.sync.dma_start(d_dbg_gath[:], gath[0][:])
                    nc.sync.dma_start(d_dbg_dwin[:], dwin[:])

                dT_ps = postps_p.tile([K, BL], F32, tag="dTps", name="dT_ps")
                nc.tensor.transpose(dT_ps[:], dwin[:, 0:K], id64)
                dT = post_p.tile([K, BL], F32, name="dT")
                nc.vector.tensor_copy(dT[:], dT_ps[:])
                nc.sync.dma_start(d_dscr[:], dT[:])
                dbc = post_p.tile([128, K * BL], F32, name="dbc")
                nc.gpsimd.dma_start(dbc[:], d_dscr[:].to_broadcast([128, K * BL]))

                gathT = [post_p.tile([128, K * BL], F32, tag=f"gathT{hc}",
                                     name=f"gathT{hc}") for hc in range(3)]
                for j in range(5):
                    for hc in range(3):
                        pt = postps2_p.tile([128, 128], F32, tag="postTp", name="pt2")
                        nc.tensor.transpose(pt[:], gath[j][:, hc * 128:(hc + 1) * 128],
                                            ident[:])
                        nc.scalar.copy(gathT[hc][:, j * 128:(j + 1) * 128], pt[:])

                wg = [post_p.tile([128, K * BL], BF16, tag=f"wg{hc}", name=f"wg{hc}")
                      for hc in range(3)]
                for hc in range(3):
                    nc.vector.tensor_tensor(wg[hc][:], gathT[hc][:], dbc[:],
                                            op=OP.mult)

                thin = [post_p.tile([128, BL], F32, tag=f"thin{hc}",
                                    name=f"thin{hc}") for hc in range(3)]
                for hc in range(3):
                    nc.vector.tensor_reduce(
                        thin[hc][:], wg[hc][:].rearrange("p (k b) -> p b k", b=BL),
                        axis=AX.X, op=OP.add)

                scaleW_sb = wf_sb[:, WF_SCALEW:WF_SCALEW + 192]
                u_ps = postps_p.tile([64, BL], F32, tag="ups", name="u_ps")
                for hc in range(3):
                    nc.tensor.matmul(u_ps[:], scaleW_sb[:, hc * 64:(hc + 1) * 64],
                                     thin[hc][:], start=(hc == 0), stop=(hc == 2))
                ru = post_p.tile([64, BL], F32, name="ru")
                nc.vector.tensor_scalar(ru[:], u_ps[:],
                                        scalar1=wf_sb[0:64, WF_SCALEB:WF_SCALEB + 1],
                                        scalar2=0.0, op0=OP.add, op1=OP.max)
                th = [post_p.tile([128, BL], F32, tag=f"th{oc}", name=f"th{oc}")
                      for oc in range(3)]
                rescaleW_sb = wf_sb[0:64, WF_RESCALEW:WF_RESCALEW + H]
                for oc in range(3):
                    v_ps = postps_p.tile([128, BL], F32, tag="vps", name="v_ps")
                    nc.tensor.matmul(v_ps[:],
                                     rescaleW_sb[:, oc * 128:(oc + 1) * 128],
                                     ru[:], start=True, stop=True)
                    nc.scalar.activation(th[oc][:], v_ps[:], AF.Tanh,
                                         bias=wf_sb[:, WF_RESCB + oc:WF_RESCB + oc + 1],
                                         scale=0.5)
                    nc.vector.tensor_scalar(th[oc][:], th[oc][:], scalar1=0.5,
                                            scalar2=0.5, op0=OP.mult, op1=OP.add)

                convT_sb = wt_sb[:, WT_CONV:WT_CONV + K * 9 * CH]
                rnnT = [post_p.tile([128, BL], F32, tag=f"rnnT{oc}",
                                    name=f"rnnT{oc}") for oc in range(3)]
                for oc in range(3):
                    cv_ps = postps2_p.tile([128, BL], F32, tag="cvps", name="cv_ps")
                    n = 0
                    for k in range(K):
                        for hc in range(3):
                            off = ((k * 3 + hc) * 3 + oc) * CH
                            nc.tensor.matmul(
                                cv_ps[:], convT_sb[:, off:off + CH],
                                wg[hc][:, k * BL:(k + 1) * BL],
                                start=(n == 0), stop=(n == 3 * K - 1))
                            n += 1
                    nc.vector.tensor_scalar(rnnT[oc][:], cv_ps[:],
                                            scalar1=wf_sb[:, WF_CONVB + oc:
                                                          WF_CONVB + oc + 1],
                                            scalar2=None, op0=OP.add)
                    nc.vector.tensor_tensor(rnnT[oc][:], rnnT[oc][:], th[oc][:],
                                            op=OP.mult)
                    nc.vector.tensor_tensor(rnnT[oc][:], rnnT[oc][:],
                                            gathT[oc][:, (K - 1) * BL:K * BL],
                                            op=OP.add)

                o_ps = postps_p.tile([BL, LAB], F32, tag="ops", name="o_ps")
                outW_sb = wf_sb[:, WF_OUTW:WF_OUTW + 3 * LAB]
                nc.tensor.matmul(o_ps[:], ones_fr[0:1, 0:BL].bitcast(F32),
                                 wf_sb[0:1, WF_OUTB:WF_OUTB + LAB],
                                 start=True, stop=False)
                for hc in range(3):
                    nc.tensor.matmul(o_ps[:], rnnT[hc][:],
                                     outW_sb[:, hc * LAB:(hc + 1) * LAB],
                                     start=False, stop=(hc == 2))
                ofin = post_p.tile([BL, LAB], F32, name="ofin")
                nc.vector.tensor_copy(ofin[:], o_ps[:])
                nc.sync.dma_start(d_out[:], ofin[:])

    _split_drain_waits(nc)
    return nc


def _split_drain_waits(nc, limit=1):
    n = 0
    for fn in nc.m.functions:
        for bb in fn.blocks:
            new_insts = []
            for inst in bb.instructions:
                si = inst.sync_info
                if si and si.on_wait and len(si.on_wait) > limit:
                    waits = list(si.on_wait)
                    for w in waits[limit:]:
                        n += 1
                        nop = mybir.InstNoOp(name=f"I-dsplit-{n}", ins=[], outs=[])
                        nop.engine = inst.engine
                        nop.sync_info = mybir.SyncInfo(on_wait=[w], on_update=[])
                        new_insts.append(nop)
                    inst.sync_info = mybir.SyncInfo(on_wait=waits[:limit],
                                                    on_update=list(si.on_update))
                new_insts.append(inst)
            bb.instructions = new_insts
    return n


def _make_inmaps(inputs, t_steps=T, ncores=NCORES):
    X = np.asarray(inputs["X"], np.float32)
    v_lengths = np.asarray(inputs["v_lengths"]).astype(np.int64)
    shared = _prep_shared(
        np.asarray(inputs["kernel_w"], np.float32), np.asarray(inputs["kernel_b"], np.float32),
        np.asarray(inputs["rec_w"], np.float32), np.asarray(inputs["rec_b"], np.float32),
        np.asarray(inputs["scale_w"], np.float32), np.asarray(inputs["scale_b"], np.float32),
        np.asarray(inputs["rescale_w"], np.float32), np.asarray(inputs["rescale_b"], np.float32),
        np.asarray(inputs["conv_w"], np.float32), np.asarray(inputs["conv_b"], np.float32),
        np.asarray(inputs["out_w"], np.float32), np.asarray(inputs["out_b"], np.float32))
    in_maps = []
    for c in range(ncores):
        bs = slice(c * BL, (c + 1) * BL)
        vl = v_lengths[bs]
        gidx = np.zeros((128, 5), np.int32)
        for p in range(128):
            for j in range(5):
                b = p % 64
                k = 2 * j + p // 64
                tb = int(vl[b]) - 1
                gidx[p, j] = (tb + k) * BL + b
        m = dict(shared)
        m["x"] = np.ascontiguousarray(X[bs, :t_steps, :])
        m["gidx"] = gidx
        in_maps.append(m)
    return in_maps


_NC_CACHE = {}


def kernel(**inputs) -> np.ndarray:
    t_steps = T
    if t_steps not in _NC_CACHE:
        _NC_CACHE[t_steps] = build_nc(t_steps)
    nc = _NC_CACHE[t_steps]
    in_maps = _make_inmaps(inputs, t_steps)
    res = run_bass_kernel_spmd(nc, in_maps, list(range(NCORES)))
    out = np.concatenate([res.results[c]["cur_out"] for c in range(NCORES)], axis=0)
    return out.astype(np.float32)


# revision 8
# speedup vs baseline: 1.0757x; 1.0757x over previous
"""Transposed-orientation rewrite of the ON-LSTM kernel.

Differences vs kernel.py (baseline):
  * The recurrence runs "transposed": gate preactivations are computed as
    xoT[gate_col, batch] via weight-stationary matmuls (lhsT = 128x128
    weight block, rhs = hT/xT [128, 64]), all in bf16 (1 cycle/row on PE).
    h state is kept as hT [h_dim, batch] so the per-step PE transposes of
    h for the next matmul disappear.
  * The constant bias row cb is injected into each PSUM accumulation with a
    rank-1 broadcast matmul (cb_slice [1,128] x ones [1,64]); accumulation
    groups are kept contiguous per step (cross-step PSUM preload corrupts
    results on HW when two groups are open in one bank).
  * Per-(batch,l) cell coefficients (s1h, s2', s3') are broadcast across the
    128 partitions with rank-1 matmuls after a small transpose.
  * The sigmoid affine (0.5x+0.5) of the f/i gates is folded into the
    coefficients: F = s1*sig(f) + s2 = (0.5 s1) tanh + (s2 + 0.5 s1).
  * All inputs packed into 4 buffers (x, wt bf16, wf f32, gidx) to cut
    per-call dispatch marshalling.

Self-contained: hardcodes all shapes; no file reads.
"""

import numpy as np
import ml_dtypes

import concourse.bass as bass
import concourse.tile as tile
from concourse import mybir
from concourse.bass_utils import run_bass_kernel_spmd
from concourse.masks import make_identity

F32 = mybir.dt.float32
F32R = mybir.dt.float32r
BF16 = mybir.dt.bfloat16
I32 = mybir.dt.int32
AF = mybir.ActivationFunctionType
OP = mybir.AluOpType
AX = mybir.AxisListType

B, T, F, H, L, K, LAB = 512, 128, 256, 384, 3, 10, 25
CH = H // L            # 128
GATES = 4 * H + 2 * L  # 1542
NG = 1536
NCORES = 8
BL = B // NCORES       # 64
PAD = K - 1

# wt (bf16) column layout
WT_WK = 0              # 2 x [128,1536]
WT_WR = 3072           # 3 x [128,1536]
WT_WKS = 7680          # 2 x [128,6]
WT_WRS = 7692          # 3 x [128,6]
WT_CONV = 7710         # [128, 11520]
WT_CB = 19230          # row 0: [1,1536] cb (t>0)
WT_CB0 = 20766         # row 0: [1,1536] cb0 (t=0)
WT_CBS = 22302         # row 0: [1,6] slot cb, [1,6] slot cb0
WT_ONES = 22314        # row 0: [1,64] ones
WT_COLS = 22378

# wf (f32) column layout
WF_SCALEW = 0          # [128,192]
WF_RESCALEW = 192      # rows 0:64 [64,384]
WF_OUTW = 576          # [128,75]
WF_RESCB = 651         # [128,3]
WF_CONVB = 654         # [128,3]
WF_SCALEB = 657        # rows 0:64 [64,1]
WF_OUTB = 658          # row 0 [1,25]
WF_SEL = 684           # rows 0:16 [16, 9*128] row-selector matrices
WF_ONES = 1836         # row 0: [1,128] ones
WF_COLS = 1964


def _gate_perm_scale():
    # gate-major col-groups: [f0 f1 f2 | i0 i1 i2 | o0 o1 o2 | ci0 ci1 ci2]
    perm = np.zeros(GATES, np.int64)
    scale = np.ones(GATES, np.float32)
    for gi in range(4):   # f, i, o, ci
        for l in range(L):
            base = (gi * 3 + l) * CH
            perm[base: base + CH] = np.arange(
                2 * L + gi * H + l * CH, 2 * L + gi * H + (l + 1) * CH)
    scale[0: 3 * 3 * CH] = 0.5   # f, i, o via sigmoid-as-tanh
    perm[NG:] = np.arange(2 * L)
    return perm, scale


def _prep_shared(kernel_w, kernel_b, rec_w, rec_b, scale_w, scale_b,
                 rescale_w, rescale_b, conv_w, conv_b, out_w, out_b):
    perm, colscale = _gate_perm_scale()

    def reorder(v):
        return (v[..., perm] * colscale).astype(np.float32)

    wpre = reorder(kernel_w[:F])                     # [256, 1542]
    wrec = reorder(rec_w[:H])                        # [384, 1542]
    cb = reorder(kernel_b + rec_b + kernel_w[F] + rec_w[H])   # [1542]
    cb0 = reorder(kernel_b + rec_b)                  # [1542] (t=0: Tint=0)

    wt = np.zeros((128, WT_COLS), np.float32)
    for fc in range(2):
        wt[:, WT_WK + fc * NG: WT_WK + (fc + 1) * NG] = wpre[fc * 128:(fc + 1) * 128, :NG]
        wt[:, WT_WKS + fc * 6: WT_WKS + (fc + 1) * 6] = wpre[fc * 128:(fc + 1) * 128, NG:]
    for hc in range(3):
        wt[:, WT_WR + hc * NG: WT_WR + (hc + 1) * NG] = wrec[hc * 128:(hc + 1) * 128, :NG]
        wt[:, WT_WRS + hc * 6: WT_WRS + (hc + 1) * 6] = wrec[hc * 128:(hc + 1) * 128, NG:]
    # conv_w [O,Hin,K] -> [128(h'), (k,hc,oc,o)]
    for k in range(K):
        for hc in range(3):
            for oc in range(3):
                blk = conv_w[oc * CH:(oc + 1) * CH, hc * CH:(hc + 1) * CH, k].T
                wt[:, WT_CONV + (((k * 3 + hc) * 3 + oc) * CH):
                   WT_CONV + (((k * 3 + hc) * 3 + oc) * CH) + CH] = blk
    wt[0, WT_CB:WT_CB + NG] = cb[:NG]
    wt[0, WT_CB0:WT_CB0 + NG] = cb0[:NG]
    wt[0, WT_CBS:WT_CBS + 6] = cb[NG:]
    wt[0, WT_CBS + 6:WT_CBS + 12] = cb0[NG:]

    wf = np.zeros((128, WF_COLS), np.float32)
    for hcc in range(3):
        wf[:, WF_SCALEW + hcc * 64: WF_SCALEW + (hcc + 1) * 64] = \
            scale_w[hcc * CH:(hcc + 1) * CH, :] / 10.0
    wf[0:64, WF_RESCALEW:WF_RESCALEW + H] = rescale_w
    for oc in range(3):
        wf[:, WF_OUTW + oc * LAB: WF_OUTW + (oc + 1) * LAB] = out_w[oc * CH:(oc + 1) * CH, :]
        wf[:, WF_RESCB + oc] = 0.5 * rescale_b[oc * CH:(oc + 1) * CH]
        wf[:, WF_CONVB + oc] = conv_b[oc * CH:(oc + 1) * CH]
    wf[0:64, WF_SCALEB] = scale_b
    wf[0, WF_OUTB:WF_OUTB + LAB] = out_b
    for j in range(9):
        wf[j, WF_SEL + j * 128:WF_SEL + (j + 1) * 128] = 1.0
    wt[0, WT_ONES:WT_ONES + BL] = 1.0
    wf[0, WF_ONES:WF_ONES + 128] = 1.0

    return dict(wt=wt.astype(ml_dtypes.bfloat16), wf=wf)


def build_nc(t_steps=T, debug=False):
    nc = bass.Bass()
    ROWS = BL * t_steps
    HS_ROWS = (t_steps + PAD) * BL

    d_x = nc.dram_tensor("x", [BL, t_steps, F], BF16, kind="ExternalInput")
    d_wt = nc.dram_tensor("wt", [128, WT_COLS], BF16, kind="ExternalInput")
    d_wf = nc.dram_tensor("wf", [128, WF_COLS], F32, kind="ExternalInput")
    d_gidx = nc.dram_tensor("gidx", [128, 5], I32, kind="ExternalInput")

    d_hseq = nc.dram_tensor("hseq", [HS_ROWS, H + 1], F32)
    d_out = nc.dram_tensor("cur_out", [BL, LAB], F32, kind="ExternalOutput")
    d_dscr = nc.dram_tensor("dscr", [1, K * BL], F32)
    d_dbg_ht = d_dbg_gath = d_dbg_dwin = None
    if debug:
        d_dbg_ht = nc.dram_tensor("dbg_ht", [t_steps * BL, H + 1], F32,
                                  kind="ExternalOutput")
        d_dbg_gath = nc.dram_tensor("dbg_gath", [128, H + 1], F32,
                                    kind="ExternalOutput")
        d_dbg_dwin = nc.dram_tensor("dbg_dwin", [BL, 16], F32,
                                    kind="ExternalOutput")

    with tile.TileContext(nc) as tc:
        with (
            tc.tile_pool(name="singles", bufs=1) as singles,
            tc.tile_pool(name="post", bufs=1) as post_p,
        ):
            ident = singles.tile([128, 128], F32)
            make_identity(nc, ident[:])
            id64 = ident[0:64, 0:64]
            ident_bf = singles.tile([128, 128], BF16)
            nc.vector.tensor_copy(ident_bf[:], ident[:])
            wt_sb = singles.tile([128, WT_COLS], BF16)
            nc.sync.dma_start(wt_sb[:], d_wt[:])
            wf_sb = singles.tile([128, WF_COLS], F32)
            nc.sync.dma_start(wf_sb[:], d_wf[:])
            gidx_sb = singles.tile([128, 5], I32)
            nc.sync.dma_start(gidx_sb[:], d_gidx[:])
            ones_bf_t = wt_sb[0:1, WT_ONES:WT_ONES + BL]
            ones_fr = singles.tile([1, 128], F32R)
            nc.sync.dma_start(ones_fr[:],
                              d_wf[0:1, WF_ONES:WF_ONES + 128].bitcast(F32R))

            WK = [wt_sb[:, WT_WK + fc * NG: WT_WK + (fc + 1) * NG] for fc in range(2)]
            WR = [wt_sb[:, WT_WR + hc * NG: WT_WR + (hc + 1) * NG] for hc in range(3)]
            WKS = [wt_sb[:, WT_WKS + fc * 6: WT_WKS + (fc + 1) * 6] for fc in range(2)]
            WRS = [wt_sb[:, WT_WRS + hc * 6: WT_WRS + (hc + 1) * 6] for hc in range(3)]
            cb_row = [wt_sb[0:1, WT_CB:WT_CB + NG],
                      wt_sb[0:1, WT_CB0:WT_CB0 + NG]]
            cbs_row = [wt_sb[0:1, WT_CBS:WT_CBS + 6],
                       wt_sb[0:1, WT_CBS + 6:WT_CBS + 12]]
            sel_sb = singles.tile([16, 9 * 128], F32R)
            nc.sync.dma_start(sel_sb[:],
                              d_wf[0:16, WF_SEL:WF_SEL + 9 * 128].bitcast(F32R))
            sel_bf = singles.tile([9, 9 * 128], BF16)
            nc.vector.tensor_copy(sel_bf[:], sel_sb[0:9, :])

            # zero hseq prefix rows
            zrow = singles.tile([128, H + 1], F32)
            nc.vector.memset(zrow[:], 0.0)
            for r0 in range(0, PAD * BL, 128):
                n = min(128, PAD * BL - r0)
                nc.sync.dma_start(d_hseq[r0:r0 + n, :], zrow[:n, :])

            # ---- phase 1: build XT (bf16, f-major) ----
            xt_sb = [singles.tile([128, ROWS], BF16, tag=f"xt{i}", name=f"xt{i}")
                     for i in range(2)]
            x_tmaj = d_x[:].rearrange("b t f -> t b f")
            with (
                tc.tile_pool(name="xrow", bufs=4) as xrow_p,
                tc.tile_pool(name="trps", bufs=4, space="PSUM") as trps_p,
            ):
                for rt in range(ROWS // 128):
                    xr = xrow_p.tile([128, F], BF16, tag="xrow", name="xr")
                    t0 = rt * 2
                    nc.sync.dma_start(xr[0:64, :], x_tmaj[t0, :, :])
                    nc.sync.dma_start(xr[64:128, :], x_tmaj[t0 + 1, :, :])
                    for fc in range(2):
                        pt = trps_p.tile([128, 128], BF16, tag="xtp", name="pt")
                        nc.tensor.transpose(pt[:], xr[:, fc * 128:(fc + 1) * 128],
                                            ident_bf[:])
                        if fc == 0:
                            nc.scalar.copy(xt_sb[fc][:, rt * 128:(rt + 1) * 128], pt[:])
                        else:
                            nc.vector.tensor_copy(xt_sb[fc][:, rt * 128:(rt + 1) * 128],
                                                  pt[:])

            # ---- phase 2: recurrence ----
            with (
                tc.tile_pool(name="xo", bufs=2, space="PSUM") as xo_p,
                tc.tile_pool(name="smallps", bufs=1, space="PSUM") as smallps_p,
                tc.tile_pool(name="gates", bufs=2) as gates_p,
                tc.tile_pool(name="state", bufs=2) as state_p,
                tc.tile_pool(name="sm", bufs=2) as sm_p,
                tc.tile_pool(name="tmp", bufs=2) as tmp_p,
            ):
                slotP = smallps_p.tile([BL, 512], F32, tag="slotP", name="slotP")
                coefP = smallps_p.tile([128, 640], F32, tag="coefP", name="coefP")
                htP = smallps_p.tile([BL, 512], F32, tag="htP", name="htP")

                hT_prev = None
                cT_prev = None

                for t in range(t_steps):
                    brow = 1 if t == 0 else 0
                    ts = slice(t * BL, (t + 1) * BL)
                    xo_cur = xo_p.tile([128, 768], F32, tag="xoT", name=f"xo{t}")
                    sslot = slotP[:, (t % 64) * 8:(t % 64) * 8 + 6]

                    # -- PE: slot then gate groups, contiguous per group --
                    nc.tensor.matmul(sslot, ones_bf_t, cbs_row[brow],
                                     start=True, stop=False)
                    for fc in range(2):
                        nc.tensor.matmul(sslot, xt_sb[fc][:, ts], WKS[fc],
                                         start=False, stop=(t == 0 and fc == 1))
                    if t > 0:
                        for hc in range(3):
                            nc.tensor.matmul(sslot,
                                             hT_prev[:, hc * BL:(hc + 1) * BL],
                                             WRS[hc], start=False, stop=(hc == 2))
                    for g in range(12):
                        dst = xo_cur[:, g * 64:(g + 1) * 64]
                        nc.tensor.matmul(dst, cb_row[brow][:, g * 128:(g + 1) * 128],
                                         ones_bf_t, start=True, stop=False)
                        for fc in range(2):
                            nc.tensor.matmul(dst, WK[fc][:, g * 128:(g + 1) * 128],
                                             xt_sb[fc][:, ts],
                                             start=False, stop=(t == 0 and fc == 1))
                        if t > 0:
                            for hc in range(3):
                                nc.tensor.matmul(dst,
                                                 WR[hc][:, g * 128:(g + 1) * 128],
                                                 hT_prev[:, hc * BL:(hc + 1) * BL],
                                                 start=False, stop=(hc == 2))

                    # -- ACT/DVE: fm/im softmax chain -> s coefficients --
                    # fm = cum-l2r softmax(fm preact), im = cum-r2l.
                    # F = s1*sig(f)+s2, I = s1*sig(i)+s3 with s1 = fm*im,
                    # s2 = fm-s1, s3 = im-s1, sigmoid folded: s1h = 0.5*s1,
                    # s2' = s2+s1h = fm-s1h, s3' = im-s1h.
                    sm = sm_p.tile([BL, 32], F32, tag="sm", name="sm")
                    coefs = sm_p.tile([BL, 16], F32, tag="coefs", name="coefs")
                    nc.scalar.activation(sm[:, 0:6], sslot, AF.Exp)
                    nc.vector.tensor_reduce(
                        sm[:, 8:10], sm[:, 0:6].rearrange("p (a b) -> p a b", b=3),
                        axis=AX.X, op=OP.add)
                    nc.vector.tensor_tensor(sm[:, 1:2], sm[:, 0:1], sm[:, 1:2],
                                            op=OP.add)   # e0+e1
                    nc.vector.tensor_tensor(sm[:, 4:5], sm[:, 5:6], sm[:, 4:5],
                                            op=OP.add)   # e4+e5
                    nc.vector.reciprocal(sm[:, 10:12], sm[:, 8:10])
                    nc.vector.memset(sm[:, 14:16], 1.0)   # fm2 = 1, im0 = 1
                    nc.vector.memset(sm[:, 28:29], 0.5)   # imh0 = 0.5
                    nc.vector.tensor_scalar(sm[:, 12:14], sm[:, 0:2],
                                            scalar1=sm[:, 10:11], scalar2=None,
                                            op0=OP.mult)          # fm0, fm1
                    nc.vector.tensor_scalar(sm[:, 16:18], sm[:, 4:6],
                                            scalar1=sm[:, 11:12], scalar2=None,
                                            op0=OP.mult)          # im1, im2
                    nc.vector.tensor_scalar(sm[:, 29:31], sm[:, 4:6],
                                            scalar1=sm[:, 11:12], scalar2=0.5,
                                            op0=OP.mult, op1=OP.mult)  # imh1, imh2
                    nc.vector.tensor_tensor(coefs[:, 0:3], sm[:, 12:15],
                                            sm[:, 28:31], op=OP.mult)  # s1h
                    nc.vector.tensor_tensor(coefs[:, 3:6], sm[:, 12:15], coefs[:, 0:3],
                                            op=OP.subtract)  # s2' = fm - s1h
                    nc.vector.tensor_tensor(coefs[:, 6:9], sm[:, 15:18], coefs[:, 0:3],
                                            op=OP.subtract)  # s3' = im - s1h

                    # -- PE: transpose coefs, broadcast rows across partitions --
                    nc.tensor.transpose(coefP[0:9, 576:640], coefs[:, 0:9], id64)
                    coefT = sm_p.tile([9, 64], BF16, tag="coefT", name="coefT")
                    nc.vector.tensor_copy(coefT[:], coefP[0:9, 576:640])
                    for j in (0, 1, 2, 6, 7, 8, 3, 4, 5):
                        nc.tensor.matmul(coefP[:, j * 64:(j + 1) * 64],
                                         sel_bf[:, j * 128:(j + 1) * 128],
                                         coefT[:], start=True, stop=True)

                    # -- ACT: gate tanh, one full-width instruction --
                    gatesT = gates_p.tile([128, 768], F32, tag="gatesT", name="gatesT")
                    nc.scalar.activation(gatesT[:], xo_cur[:], AF.Tanh)
                    thf = gatesT[:, 0:192]
                    thi = gatesT[:, 192:384]
                    tho = gatesT[:, 384:576]
                    ci = gatesT[:, 576:768]
                    s1h = coefP[:, 0:192]
                    s2p = coefP[:, 192:384]
                    s3p = coefP[:, 384:576]

                    # -- cell update, full-width [128,192] ops --
                    # c = s1h.(thi.ci + thf.c_prev) + (s3'.ci + s2'.c_prev)
                    # spine ops on DVE (258ns); Pool feeds the side products.
                    W3 = 3 * BL
                    a = tmp_p.tile([128, W3], F32, tag="a", name="a")
                    nc.vector.tensor_tensor(a[:], thi, ci, op=OP.mult)
                    if t > 0:
                        b_ = tmp_p.tile([128, W3], F32, tag="b", name="b_")
                        nc.gpsimd.tensor_tensor(b_[:], thf, cT_prev[:], op=OP.mult)
                        nc.vector.tensor_tensor(a[:], a[:], b_[:], op=OP.add)
                    m = tmp_p.tile([128, W3], F32, tag="m", name="m")
                    nc.vector.tensor_tensor(m[:], s1h, a[:], op=OP.mult)
                    e = tmp_p.tile([128, W3], F32, tag="e", name="e")
                    nc.vector.tensor_tensor(e[:], s3p, ci, op=OP.mult)
                    if t > 0:
                        f2 = tmp_p.tile([128, W3], F32, tag="f", name="f2")
                        nc.vector.tensor_tensor(f2[:], s2p, cT_prev[:], op=OP.mult)
                        nc.gpsimd.tensor_tensor(e[:], e[:], f2[:], op=OP.add)
                    c_new = state_p.tile([128, W3], F32, tag="c", name="c_new")
                    nc.vector.tensor_tensor(c_new[:], m[:], e[:], op=OP.add)
                    tc_ = tmp_p.tile([128, W3], F32, tag="tc", name="tc_")
                    nc.scalar.activation(tc_[:], c_new[:], AF.Tanh)
                    oaf = tmp_p.tile([128, W3], F32, tag="oaf", name="oaf")
                    nc.gpsimd.tensor_scalar(oaf[:], tho, scalar1=0.5, scalar2=0.5,
                                            op0=OP.mult, op1=OP.add)
                    hT_new = state_p.tile([128, W3], BF16, tag="hTn", name="hT_new")
                    nc.vector.tensor_tensor(hT_new[:], tc_[:], oaf[:], op=OP.mult)
                    hTf = state_p.tile([128, W3], F32, tag="hTf", name="hTf")
                    nc.gpsimd.tensor_tensor(hTf[:], tc_[:], oaf[:], op=OP.mult)

                    # -- dist for the window head --
                    ht_sb = sm_p.tile([BL, H + 1], F32, tag="ht_sb", name="ht_sb")
                    nc.vector.tensor_tensor(sm[:, 27:28], sm[:, 12:13], sm[:, 13:14],
                                            op=OP.add)
                    nc.vector.tensor_scalar(
                        ht_sb[:, H:H + 1], sm[:, 27:28],
                        scalar1=-1.0 / 3.0, scalar2=2.0 / 3.0, op0=OP.mult, op1=OP.add)

                    # -- PE: h transposes for the hseq store --
                    for l in range(3):
                        nc.tensor.transpose(htP[:, l * 128:(l + 1) * 128],
                                            hTf[:, l * BL:(l + 1) * BL], ident[:])
                    nc.vector.tensor_copy(ht_sb[:, 0:H], htP[:, 0:H])
                    nc.sync.dma_start(d_hseq[(t + PAD) * BL:(t + PAD + 1) * BL, :],
                                      ht_sb[:])
                    if debug:
                        nc.sync.dma_start(d_dbg_ht[t * BL:(t + 1) * BL, :], ht_sb[:])

                    hT_prev = hT_new[:]
                    cT_prev = c_new[:]

            # ---- phase 3: windowed head at t_b only (baseline logic) ----
            with (
                tc.tile_pool(name="postps", bufs=1, space="PSUM") as postps_p,
                tc.tile_pool(name="postps2", bufs=2, space="PSUM") as postps2_p,
            ):
                gath = [post_p.tile([128, H + 1], F32, tag=f"gath{j}",
                                    name=f"gath{j}") for j in range(5)]
                for j in range(5):
                    nc.gpsimd.indirect_dma_start(
                        out=gath[j][:], out_offset=None, in_=d_hseq[:],
                        in_offset=bass.IndirectOffsetOnAxis(ap=gidx_sb[:, j:j + 1],
                                                            axis=0))

                ww = post_p.tile([BL, 16], F32, name="ww")
                cum = post_p.tile([BL, 64], F32, name="cum")
                for k in range(K):
                    j, par = k // 2, k % 2
                    nc.vector.tensor_copy(ww[:, k:k + 1],
                                          gath[j][par * 64:par * 64 + 64, H:H + 1])
                nc.vector.tensor_copy(cum[:, 0:1], ww[:, 0:1])
                nc.vector.tensor_tensor(cum[:, 1:10], ww[:, 1:10], ww[:, 0:9],
                                        op=OP.add)
                nc.vector.tensor_copy(cum[:, 16:18], cum[:, 0:2])
                nc.vector.tensor_tensor(cum[:, 18:26], cum[:, 2:10], cum[:, 0:8],
                                        op=OP.add)
                nc.vector.tensor_copy(cum[:, 32:36], cum[:, 16:20])
                nc.vector.tensor_tensor(cum[:, 36:42], cum[:, 20:26], cum[:, 16:22],
                                        op=OP.add)
                nc.vector.tensor_copy(cum[:, 48:56], cum[:, 32:40])
                nc.vector.tensor_tensor(cum[:, 56:58], cum[:, 40:42], cum[:, 32:34],
                                        op=OP.add)
                nmx = post_p.tile([BL, 4], F32, name="nmx")
                nc.vector.tensor_reduce(nmx[:, 0:1], cum[:, 48:58], axis=AX.X,
                                        op=OP.max, negate=True)
                dwin = post_p.tile([BL, 16], F32, name="dwin")
                nc.scalar.activation(dwin[:, 0:10], cum[:, 48:58], AF.Exp,
                                     bias=nmx[:, 0:1], scale=1.0)
                nc.vector.tensor_reduce(nmx[:, 1:2], dwin[:, 0:10], axis=AX.X,
                                        op=OP.add)
                nc.vector.reciprocal(nmx[:, 2:3], nmx[:, 1:2])
                nc.vector.tensor_scalar(dwin[:, 0:10], dwin[:, 0:10],
                                        scalar1=nmx[:, 2:3], scalar2=None,
                                        op0=OP.mult)
                if debug:
                    nc.sync.dma_start(d_dbg_gath[:], gath[0][:])
                    nc.sync.dma_start(d_dbg_dwin[:], dwin[:])

                dT_ps = postps_p.tile([K, BL], F32, tag="dTps", name="dT_ps")
                nc.tensor.transpose(dT_ps[:], dwin[:, 0:K], id64)
                dT = post_p.tile([K, BL], F32, name="dT")
                nc.vector.tensor_copy(dT[:], dT_ps[:])
                nc.sync.dma_start(d_dscr[:], dT[:])
                dbc = post_p.tile([128, K * BL], F32, name="dbc")
                nc.gpsimd.dma_start(dbc[:], d_dscr[:].to_broadcast([128, K * BL]))

                gathT = [post_p.tile([128, K * BL], F32, tag=f"gathT{hc}",
                                     name=f"gathT{hc}") for hc in range(3)]
                for j in range(5):
                    for hc in range(3):
                        pt = postps2_p.tile([128, 128], F32, tag="postTp", name="pt2")
                        nc.tensor.transpose(pt[:], gath[j][:, hc * 128:(hc + 1) * 128],
                                            ident[:])
                        nc.scalar.copy(gathT[hc][:, j * 128:(j + 1) * 128], pt[:])

                wg = [post_p.tile([128, K * BL], BF16, tag=f"wg{hc}", name=f"wg{hc}")
                      for hc in range(3)]
                for hc in range(3):
                    nc.vector.tensor_tensor(wg[hc][:], gathT[hc][:], dbc[:],
                                            op=OP.mult)

                thin = [post_p.tile([128, BL], F32, tag=f"thin{hc}",
                                    name=f"thin{hc}") for hc in range(3)]
                for hc in range(3):
                    nc.vector.tensor_reduce(
                        thin[hc][:], wg[hc][:].rearrange("p (k b) -> p b k", b=BL),
                        axis=AX.X, op=OP.add)

                scaleW_sb = wf_sb[:, WF_SCALEW:WF_SCALEW + 192]
                u_ps = postps_p.tile([64, BL], F32, tag="ups", name="u_ps")
                for hc in range(3):
                    nc.tensor.matmul(u_ps[:], scaleW_sb[:, hc * 64:(hc + 1) * 64],
                                     thin[hc][:], start=(hc == 0), stop=(hc == 2))
                ru = post_p.tile([64, BL], F32, name="ru")
                nc.vector.tensor_scalar(ru[:], u_ps[:],
                                        scalar1=wf_sb[0:64, WF_SCALEB:WF_SCALEB + 1],
                                        scalar2=0.0, op0=OP.add, op1=OP.max)
                th = [post_p.tile([128, BL], F32, tag=f"th{oc}", name=f"th{oc}")
                      for oc in range(3)]
                rescaleW_sb = wf_sb[0:64, WF_RESCALEW:WF_RESCALEW + H]
                for oc in range(3):
                    v_ps = postps_p.tile([128, BL], F32, tag="vps", name="v_ps")
                    nc.tensor.matmul(v_ps[:],
                                     rescaleW_sb[:, oc * 128:(oc + 1) * 128],
                                     ru[:], start=True, stop=True)
                    nc.scalar.activation(th[oc][:], v_ps[:], AF.Tanh,
                                         bias=wf_sb[:, WF_RESCB + oc:WF_RESCB + oc + 1],
                                         scale=0.5)
                    nc.vector.tensor_scalar(th[oc][:], th[oc][:], scalar1=0.5,
                                            scalar2=0.5, op0=OP.mult, op1=OP.add)

                convT_sb = wt_sb[:, WT_CONV:WT_CONV + K * 9 * CH]
                rnnT = [post_p.tile([128, BL], F32, tag=f"rnnT{oc}",
                                    name=f"rnnT{oc}") for oc in range(3)]
                for oc in range(3):
                    cv_ps = postps2_p.tile([128, BL], F32, tag="cvps", name="cv_ps")
                    n = 0
                    for k in range(K):
                        for hc in range(3):
                            off = ((k * 3 + hc) * 3 + oc) * CH
                            nc.tensor.matmul(
                                cv_ps[:], convT_sb[:, off:off + CH],
                                wg[hc][:, k * BL:(k + 1) * BL],
                                start=(n == 0), stop=(n == 3 * K - 1))
                            n += 1
                    nc.vector.tensor_scalar(rnnT[oc][:], cv_ps[:],
                                            scalar1=wf_sb[:, WF_CONVB + oc:
                                                          WF_CONVB + oc + 1],
                                            scalar2=None, op0=OP.add)
                    nc.vector.tensor_tensor(rnnT[oc][:], rnnT[oc][:], th[oc][:],
                                            op=OP.mult)
                    nc.vector.tensor_tensor(rnnT[oc][:], rnnT[oc][:],
                                            gathT[oc][:, (K - 1) * BL:K * BL],
                                            op=OP.add)

                o_ps = postps_p.tile([BL, LAB], F32, tag="ops", name="o_ps")
                outW_sb = wf_sb[:, WF_OUTW:WF_OUTW + 3 * LAB]
                nc.tensor.matmul(o_ps[:], ones_fr[0:1, 0:BL].bitcast(F32),
                                 wf_sb[0:1, WF_OUTB:WF_OUTB + LAB],
                                 start=True, stop=False)
                for hc in range(3):
                    nc.tensor.matmul(o_ps[:], rnnT[hc][:],
                                     outW_sb[:, hc * LAB:(hc + 1) * LAB],
                                     start=False, stop=(hc == 2))
                ofin = post_p.tile([BL, LAB], F32, name="ofin")
                nc.vector.tensor_copy(ofin[:], o_ps[:])
                nc.sync.dma_start(d_out[:], ofin[:])

    _split_drain_waits(nc)
    return nc


def _split_drain_waits(nc, limit=1):
    n = 0
    for fn in nc.m.functions:
        for bb in fn.blocks:
            new_insts = []
            for inst in bb.instructions:
                si = inst.sync_info
                if si and si.on_wait and len(si.on_wait) > limit:
                    waits = list(si.on_wait)
                    for w in waits[limit:]:
                        n += 1
                        nop = mybir.InstNoOp(name=f"I-dsplit-{n}", ins=[], outs=[])
                        nop.engine = inst.engine
                        nop.sync_info = mybir.SyncInfo(on_wait=[w], on_update=[])
                        new_insts.append(nop)
                    inst.sync_info = mybir.SyncInfo(on_wait=waits[:limit],
                                                    on_update=list(si.on_update))
                new_insts.append(inst)
            bb.instructions = new_insts
    return n


def _make_inmaps(inputs, t_steps=T, ncores=NCORES):
    X = np.asarray(inputs["X"], np.float32)
    v_lengths = np.asarray(inputs["v_lengths"]).astype(np.int64)
    shared = _prep_shared(
        np.asarray(inputs["kernel_w"], np.float32), np.asarray(inputs["kernel_b"], np.float32),
        np.asarray(inputs["rec_w"], np.float32), np.asarray(inputs["rec_b"], np.float32),
        np.asarray(inputs["scale_w"], np.float32), np.asarray(inputs["scale_b"], np.float32),
        np.asarray(inputs["rescale_w"], np.float32), np.asarray(inputs["rescale_b"], np.float32),
        np.asarray(inputs["conv_w"], np.float32), np.asarray(inputs["conv_b"], np.float32),
        np.asarray(inputs["out_w"], np.float32), np.asarray(inputs["out_b"], np.float32))
    in_maps = []
    for c in range(ncores):
        bs = slice(c * BL, (c + 1) * BL)
        vl = v_lengths[bs]
        gidx = np.zeros((128, 5), np.int32)
        for p in range(128):
            for j in range(5):
                b = p % 64
                k = 2 * j + p // 64
                tb = int(vl[b]) - 1
                gidx[p, j] = (tb + k) * BL + b
        m = dict(shared)
        m["x"] = np.ascontiguousarray(X[bs, :t_steps, :]).astype(ml_dtypes.bfloat16)
        m["gidx"] = gidx
        in_maps.append(m)
    return in_maps


_NC_CACHE = {}


def kernel(**inputs) -> np.ndarray:
    t_steps = T
    if t_steps not in _NC_CACHE:
        _NC_CACHE[t_steps] = build_nc(t_steps)
    nc = _NC_CACHE[t_steps]
    in_maps = _make_inmaps(inputs, t_steps)
    res = run_bass_kernel_spmd(nc, in_maps, list(range(NCORES)))
    out = np.concatenate([res.results[c]["cur_out"] for c in range(NCORES)], axis=0)
    return out.astype(np.float32)
